# revision 3
# baseline (speedup 1.0000x reference)
"""Bahdanau-style attention kernel for Trainium2 (8 NeuronCores, batch-parallel).

Computes, for B=16, S=4096, H=512:
    hid  = hidden @ W_attn[:H] + b_attn                       (B, H)
    en   = tanh(hid[:,None,:] + enc @ W_attn[H:])             (B, S, H)
    lg   = en @ v                                             (B, S, 1)
    w    = softmax(lg, axis=1)
    ctx  = w^T @ enc                                          (B, 1, 2H)

Sharding: data-parallel over batch, 2 batches per core. Per core:
  - The big projection matmul streams enc through the PE. The first
    2*F8PAIRS e-tiles of the contraction run as fp8e4 DoubleRow matmuls
    (2 e-tiles per instruction, ~2x rate); the rest run fp16. A second
    fp16 e-major copy of enc feeds the context accumulation, so fp8
    error only perturbs the softmax weights, not the averaged values.
  - DRAM layouts are packed host-side so each (group, stream) transfer
    is one DMA with 8KB-contiguous per-partition runs (s-block-major).
  - tanh(+hid bias) fused on the scalar engine (per-partition bias).
  - logits computed with v replicated across 128 partitions as the
    stationary operand, so exp(logits) lands pre-broadcast for the
    context weighting; exp's accum_out yields the softmax normalizer.
  - softmax max-subtraction is replaced by a constant shift |v|_1 (a
    hard bound on |logit| since tanh in [-1,1]), exact after
    normalization. Weights are stored bf16 (fp32 exponent range - the
    shifted exps are ~e-18 and would underflow fp16).
  - context = sum_s w_s * enc[e, s] via scalar_tensor_tensor's fused
    accumulate (reduction along free dim), split between the vector
    engine (e-tiles < 8-GP_ET) and the otherwise-idle GPSIMD engine
    (last GP_ET e-tiles), each with its own accumulator slots.
No cross-core communication; output gathered on host.
"""

import os
import numpy as np
import ml_dtypes
from contextlib import ExitStack

import concourse.bacc as bacc
import concourse.tile as tile
from concourse import mybir
from concourse.bass_utils import run_bass_kernel_spmd

F32 = mybir.dt.float32
F16 = mybir.dt.float16
BF16 = mybir.dt.bfloat16
F8 = mybir.dt.float8e4
E4M3 = ml_dtypes.float8_e4m3

B, S, H = 16, 4096, 512
E = 2 * H                      # 1024 encoder feature dim
NCORES = 8
BPC = B // NCORES              # batches per core = 2
ET = E // 128                  # 8 e-tiles
HT = H // 128                  # 4 h-tiles
SBLK = 512                     # s-block width
NSB = S // SBLK                # 8 s-blocks per batch
KT = H // 128                  # 4 k-tiles for the hidden projection

F8PAIRS = int(os.environ.get("ATTN_F8PAIRS", "4"))   # e-tile pairs in fp8-DR
GROUP_SB = int(os.environ.get("ATTN_GROUP_SB", "2"))
GP_ET = int(os.environ.get("ATTN_GP_ET", "3"))       # ctx e-tiles on gpsimd
E8T = 2 * F8PAIRS              # e-tiles fed as fp8
EW = ET * SBLK                 # fp16 elems per (partition, s-block)
E8W = E8T * SBLK               # fp8 elems per (partition, s-block)

TRACE = False          # set by test harness; harness-default off
LAST_RESULTS = None    # last BassKernelResults (for profiling in test.py)

_NC_CACHE = {}


def _groups():
    groups = []
    pos = 0
    while pos < NSB - 2:
        groups.append(list(range(pos, pos + GROUP_SB)))
        pos += GROUP_SB
    while pos < NSB:
        groups.append([pos])
        pos += 1
    return groups


def _build():
    nc = bacc.Bacc("TRN2", target_bir_lowering=False, debug=False)

    CW = (KT + 1) + KT * BPC            # bshift | hidT, packed (128, CW) f32
    # s-block-major packed layouts: [b, sb, p, t*SBLK+s]
    encT = nc.dram_tensor("encT", [BPC, NSB, 128, EW], F16,
                          kind="ExternalInput").ap()
    if E8T:
        encT8 = nc.dram_tensor("encT8", [BPC, NSB, 128, E8W], F8,
                               kind="ExternalInput").ap()
        We8_d = nc.dram_tensor(
            "We8", [128, E8T * H], F8, kind="ExternalInput").ap()
    We_d = nc.dram_tensor("We", [128, ET * H], F16, kind="ExternalInput").ap()
    V_d = nc.dram_tensor("V128", [128, HT * 128], F16, kind="ExternalInput").ap()
    Wh_d = nc.dram_tensor("Wh16", [128, KT * H], F16, kind="ExternalInput").ap()
    cst_d = nc.dram_tensor("consts", [128, CW], F32, kind="ExternalInput").ap()
    ctx_d = nc.dram_tensor("ctx", [BPC, E], F32, kind="ExternalOutput").ap()

    groups = _groups()
    NG = len(groups)
    DVE_ET = ET - GP_ET

    with tile.TileContext(nc) as tc, ExitStack() as ctx:
        cpool = ctx.enter_context(tc.tile_pool(name="consts", bufs=1))
        epool = ctx.enter_context(tc.tile_pool(name="enc", bufs=3))
        e8pool = ctx.enter_context(tc.tile_pool(name="enc8", bufs=3))
        tpool = ctx.enter_context(tc.tile_pool(name="tanh", bufs=2))
        wpool = ctx.enter_context(tc.tile_pool(name="wexp", bufs=2))
        jpool = ctx.enter_context(tc.tile_pool(name="junk", bufs=2))
        spool = ctx.enter_context(tc.tile_pool(name="stats", bufs=1))
        proj_bufs = 1 if GROUP_SB >= 4 else 2
        pp = ctx.enter_context(tc.tile_pool(name="pproj", bufs=proj_bufs, space="PSUM"))
        pl = ctx.enter_context(tc.tile_pool(name="plog", bufs=2, space="PSUM"))
        ph_pool = ctx.enter_context(tc.tile_pool(name="phid", bufs=1, space="PSUM"))

        # ---- PE warm-up: dummy matmuls while DMAs land (HAM -> K=8/8) ----
        wlhs = cpool.tile([128, 128], F16)
        wrhs = cpool.tile([128, 256], F16)
        nc.vector.memset(wlhs[:], 0.0)
        nc.vector.memset(wrhs[:], 0.0)
        wps = ph_pool.tile([128, 256], F32, name="warm", tag="ph")
        for _ in range(18):
            nc.tensor.matmul(wps[:], wlhs[:], wrhs[:], start=True, stop=True)

        # ---- constants: packed DMAs (small gate-the-warmup one first) ----
        cst_sb = cpool.tile([128, CW], F32)
        nc.scalar.dma_start(cst_sb[:], cst_d)
        if E8T:
            We8_sb = cpool.tile([128, E8T * H], F8)
            nc.scalar.dma_start(We8_sb[:], We8_d)
            We8_v = We8_sb[:].rearrange("p (t c) -> p t c", t=E8T)
        We_sb = cpool.tile([128, ET * H], F16)          # per e-tile: (128, 512)
        nc.scalar.dma_start(We_sb[:, 0:2 * H], We_d[:, 0:2 * H])
        Wh_sb = cpool.tile([128, KT * H], F16)
        nc.scalar.dma_start(Wh_sb[:], Wh_d)
        nc.scalar.dma_start(We_sb[:, 2 * H:], We_d[:, 2 * H:])
        V_sb = cpool.tile([128, HT * 128], F16)
        nc.scalar.dma_start(V_sb[:], V_d)
        bsh_sb = cst_sb[:, 0:KT + 1]
        hidT16 = cpool.tile([128, KT * BPC], F16)
        nc.vector.tensor_copy(hidT16[:], cst_sb[:, KT + 1:KT + 1 + KT * BPC])

        # ---- hidden projection: hid_sb[:, h*BPC + b] = (hidden @ Wh + b)[b, h-tile]
        hid_sb = spool.tile([128, HT * BPC], F32)
        for h in range(HT):
            ph = ph_pool.tile([128, BPC], F32, name="ph")
            for k in range(KT):
                nc.tensor.matmul(
                    ph[:],
                    Wh_sb[:, k * H + h * 128: k * H + (h + 1) * 128],
                    hidT16[:, k * BPC:(k + 1) * BPC],
                    start=(k == 0), stop=(k == KT - 1),
                )
            nc.vector.tensor_scalar_add(
                hid_sb[:, h * BPC:(h + 1) * BPC], ph[:], bsh_sb[:, h:h + 1])

        # ---- stats accumulators (per-group ctx partials; DVE/GP separate)
        zslots = spool.tile([128, BPC * NSB], F32)
        cslots = spool.tile([128, BPC * DVE_ET * NG], F32)
        if GP_ET:
            gslots = spool.tile([128, BPC * GP_ET * NG], F32)

        ctx_red = spool.tile([128, BPC * ET], F32)
        zred = spool.tile([128, BPC], F32)
        zrec = spool.tile([128, BPC], F32)
        ctx_fin = spool.tile([128, BPC * ET], F32)

        for b in range(BPC):
            for g, sbs in enumerate(groups):
                gsb = len(sbs)
                gw = gsb * SBLK
                g0 = sbs[0]
                # fp8 stream feeds the PE as early as possible
                if E8T:
                    enc8g = e8pool.tile([128, GROUP_SB * E8W], F8,
                                        name="enc8g", tag="enc8g")
                    if b == 0 and g == 0:
                        # split so the first h-iteration can start earlier
                        for i in range(gsb):
                            for t0, tn in ((0, E8T // 2), (E8T // 2, E8T // 2)):
                                nc.sync.dma_start(
                                    enc8g[:, i * E8W + t0 * SBLK:
                                          i * E8W + (t0 + tn) * SBLK],
                                    encT8[b, sbs[i], :,
                                          t0 * SBLK:(t0 + tn) * SBLK],
                                )
                    else:
                        nc.sync.dma_start(
                            enc8g[:, 0:gsb * E8W].rearrange(
                                "p (i f) -> p i f", i=gsb),
                            encT8[b].rearrange("s p f -> p s f")[
                                :, g0:g0 + gsb, :],
                        )
                    enc8g_v = enc8g[:, 0:gsb * E8W].rearrange(
                        "p (i t s) -> p i t s", i=gsb, t=E8T)
                # fp16 stream feeds the DVE/GP ctx pass (and fp16 matmuls)
                encg = epool.tile([128, GROUP_SB * EW], F16, name="encg",
                                  tag="encg")
                nc.scalar.dma_start(
                    encg[:, 0:gsb * EW].rearrange("p (i f) -> p i f", i=gsb),
                    encT[b].rearrange("s p f -> p s f")[:, g0:g0 + gsb, :],
                )
                encg_v = encg[:, 0:gsb * EW].rearrange(
                    "p (i t s) -> p i t s", i=gsb, t=ET)

                # big projection + tanh, h-tile at a time
                tanh_t = {}
                for h in range(HT):
                    proj = {}
                    for i in range(gsb):
                        proj[i] = pp.tile([128, SBLK], F32, name=f"proj_{i}")
                    for ep in range(F8PAIRS):
                        lhsT = We8_v[:, 2 * ep:2 * ep + 2,
                                     h * 128:(h + 1) * 128]
                        for i in range(gsb):
                            nc.tensor.matmul(
                                proj[i][:], lhsT,
                                enc8g_v[:, i, 2 * ep:2 * ep + 2, :],
                                start=(ep == 0),
                                stop=(ep == F8PAIRS - 1 and E8T == ET),
                                perf_mode=mybir.MatmulPerfMode.DoubleRow,
                            )
                    for e in range(E8T, ET):
                        lhs = We_sb[:, e * H + h * 128: e * H + (h + 1) * 128]
                        for i in range(gsb):
                            nc.tensor.matmul(
                                proj[i][:], lhs, encg_v[:, i, e, :],
                                start=(e == 0), stop=(e == ET - 1),
                            )
                    for i in range(gsb):
                        tt = tpool.tile([128, SBLK], F16, name=f"tanh_{h}_{i}")
                        nc.scalar.activation(
                            tt[:], proj[i][:], mybir.ActivationFunctionType.Tanh,
                            bias=hid_sb[:, h * BPC + b: h * BPC + b + 1],
                        )
                        tanh_t[(h, i)] = tt

                # logits (broadcast across partitions) + exp + Z accum
                wg = wpool.tile([128, GROUP_SB * SBLK], BF16, name="wg")
                lg = {}
                for i in range(gsb):
                    lg[i] = pl.tile([128, SBLK], F32, name=f"logits_{i}",
                                    bufs=1)
                for h in range(HT):
                    for i in range(gsb):
                        nc.tensor.matmul(
                            lg[i][:], V_sb[:, h * 128:(h + 1) * 128],
                            tanh_t[(h, i)][:],
                            start=(h == 0), stop=(h == HT - 1),
                        )
                for i, sb in enumerate(sbs):
                    nc.scalar.activation(
                        wg[:, i * SBLK:(i + 1) * SBLK], lg[i][:],
                        mybir.ActivationFunctionType.Exp,
                        bias=bsh_sb[:, KT:KT + 1],
                        accum_out=zslots[:, b * NSB + sb: b * NSB + sb + 1],
                    )
                wg_v = wg[:, 0:gw].rearrange("p (i s) -> p i s", i=gsb)

                # context accumulation: fused multiply+accumulate per e-tile,
                # whole group in one 3D-AP op; split DVE / GPSIMD by e-tile.
                for e in range(ET):
                    if e < DVE_ET:
                        eng, name = nc.vector, "junk"
                        col = (b * DVE_ET + e) * NG + g
                        acc = cslots
                    else:
                        eng, name = nc.gpsimd, "gjunk"
                        col = (b * GP_ET + (e - DVE_ET)) * NG + g
                        acc = gslots
                    jt = jpool.tile([128, GROUP_SB * SBLK], F16, name=name)
                    eng.scalar_tensor_tensor(
                        jt[:, 0:gw].rearrange("p (i s) -> p i s", i=gsb),
                        encg_v[:, :, e, :], 1.0, wg_v,
                        mybir.AluOpType.mult, mybir.AluOpType.mult,
                        accum_out=acc[:, col:col + 1],
                    )

            # finalize this batch: ctx = (sum_g ctx_partial) / Z
            nc.vector.tensor_reduce(
                ctx_red[:, b * ET:b * ET + DVE_ET],
                cslots[:, b * DVE_ET * NG:(b + 1) * DVE_ET * NG].rearrange(
                    "p (q s) -> p q s", s=NG),
                axis=mybir.AxisListType.X, op=mybir.AluOpType.add)
            if GP_ET:
                nc.vector.tensor_reduce(
                    ctx_red[:, b * ET + DVE_ET:(b + 1) * ET],
                    gslots[:, b * GP_ET * NG:(b + 1) * GP_ET * NG].rearrange(
                        "p (q s) -> p q s", s=NG),
                    axis=mybir.AxisListType.X, op=mybir.AluOpType.add)
            nc.vector.tensor_reduce(
                zred[:, b:b + 1],
                zslots[:, b * NSB:(b + 1) * NSB].rearrange(
                    "p (q s) -> p q s", s=NSB),
                axis=mybir.AxisListType.X, op=mybir.AluOpType.add)
            nc.vector.reciprocal(zrec[:, b:b + 1], zred[:, b:b + 1])
            nc.vector.tensor_scalar_mul(
                ctx_fin[:, b * ET:(b + 1) * ET],
                ctx_red[:, b * ET:(b + 1) * ET], zrec[:, b:b + 1])
            nc.sync.dma_start(
                ctx_d[b].rearrange("(e p) -> p e", p=128),
                ctx_fin[:, b * ET:(b + 1) * ET])

    nc.compile()
    return nc


def kernel(hidden, encoder_outputs, W_attn, b_attn, v):
    global LAST_RESULTS
    hidden = np.asarray(hidden, dtype=np.float32)
    encoder_outputs = np.asarray(encoder_outputs, dtype=np.float32)
    W_attn = np.asarray(W_attn, dtype=np.float32)
    b_attn = np.asarray(b_attn, dtype=np.float32)
    v = np.asarray(v, dtype=np.float32)

    key = (F8PAIRS, GROUP_SB, GP_ET)
    if key not in _NC_CACHE:
        _NC_CACHE[key] = _build()
    nc = _NC_CACHE[key]

    # SBUF-layout packed constants (partition dim = 128 rows)
    WeT = W_attn[H:].reshape(ET, 128, H).transpose(1, 0, 2).reshape(128, ET * H)
    We_f = np.ascontiguousarray(WeT.astype(np.float16))
    V128 = np.ascontiguousarray(np.broadcast_to(
        v.reshape(HT, 128, 1).transpose(1, 0, 2), (128, HT, 128)
    ).reshape(128, HT * 128).astype(np.float16))
    Wh16 = np.ascontiguousarray(W_attn[:H].reshape(KT, 128, H).transpose(
        1, 0, 2).reshape(128, KT * H).astype(np.float16))
    shift = float(np.abs(v).sum())
    bsh = np.zeros((128, KT + 1), dtype=np.float32)
    bsh[:, :KT] = b_attn.reshape(KT, 128).T
    bsh[:, KT] = -shift
    if E8T:
        We8 = np.ascontiguousarray(WeT[:, 0:E8T * H].astype(E4M3))

    in_maps = []
    for c in range(NCORES):
        sl = slice(c * BPC, (c + 1) * BPC)
        # [b, sb, p, t, s] s-block-major pack (8KB contiguous per partition)
        encP = encoder_outputs[sl].reshape(
            BPC, NSB, SBLK, ET, 128).transpose(0, 1, 4, 3, 2)
        hidT_pack = np.ascontiguousarray(
            hidden[sl].T.reshape(KT, 128, BPC).transpose(1, 0, 2)
        ).reshape(128, KT * BPC)
        consts = np.ascontiguousarray(
            np.concatenate([bsh, hidT_pack], axis=1, dtype=np.float32))
        m = {
            "encT": np.ascontiguousarray(encP.astype(np.float16)).reshape(
                BPC, NSB, 128, EW),
            "We": We_f, "V128": V128, "consts": consts, "Wh16": Wh16,
        }
        if E8T:
            m["encT8"] = np.ascontiguousarray(
                encP[:, :, :, 0:E8T, :].astype(E4M3)).reshape(
                    BPC, NSB, 128, E8W)
            m["We8"] = We8
        in_maps.append(m)

    res = run_bass_kernel_spmd(
        nc, in_maps, core_ids=list(range(NCORES)), trace=TRACE)
    LAST_RESULTS = res

    out = np.empty((B, 1, E), dtype=np.float32)
    for c in range(NCORES):
        out[c * BPC:(c + 1) * BPC, 0, :] = res.results[c]["ctx"]
    return out


# revision 5
# speedup vs baseline: 1.2080x; 1.2080x over previous
"""Bahdanau-style attention kernel for Trainium2 (8 NeuronCores, batch-parallel).

Computes, for B=16, S=4096, H=512:
    hid  = hidden @ W_attn[:H] + b_attn                       (B, H)
    en   = tanh(hid[:,None,:] + enc @ W_attn[H:])             (B, S, H)
    lg   = en @ v                                             (B, S, 1)
    w    = softmax(lg, axis=1)
    ctx  = w^T @ enc                                          (B, 1, 2H)

Sharding: data-parallel over batch, 2 batches per core. Per core:
  - The big projection matmul streams enc through the PE. The first
    2*F8PAIRS e-tiles of the contraction run as fp8e4 DoubleRow matmuls
    (2 e-tiles per instruction, ~2x rate); the rest run fp16. A second
    fp16 e-major copy of enc feeds the context accumulation, so fp8
    error only perturbs the softmax weights, not the averaged values.
  - DRAM layouts are packed host-side so each (group, stream) transfer
    is one DMA with 8KB-contiguous per-partition runs (s-block-major).
  - tanh(+hid bias) fused on the scalar engine (per-partition bias).
  - logits computed with v replicated across 128 partitions as the
    stationary operand, so exp(logits) lands pre-broadcast for the
    context weighting; exp's accum_out yields the softmax normalizer.
  - softmax max-subtraction is replaced by a constant shift |v|_1 (a
    hard bound on |logit| since tanh in [-1,1]), exact after
    normalization. Weights are stored bf16 (fp32 exponent range - the
    shifted exps are ~e-18 and would underflow fp16).
  - context = sum_s w_s * enc[e, s] via scalar_tensor_tensor's fused
    accumulate (reduction along free dim), split between the vector
    engine (e-tiles < 8-GP_ET) and the otherwise-idle GPSIMD engine
    (last GP_ET e-tiles), each with its own accumulator slots.
No cross-core communication; output gathered on host.
"""

import os
import numpy as np
import ml_dtypes
from contextlib import ExitStack

import concourse.bacc as bacc
import concourse.tile as tile
from concourse import mybir
from concourse.bass_utils import run_bass_kernel_spmd

F32 = mybir.dt.float32
F16 = mybir.dt.float16
BF16 = mybir.dt.bfloat16
F8 = mybir.dt.float8e4
E4M3 = ml_dtypes.float8_e4m3

B, S, H = 16, 4096, 512
E = 2 * H                      # 1024 encoder feature dim
NCORES = 8
BPC = B // NCORES              # batches per core = 2
ET = E // 128                  # 8 e-tiles
HT = H // 128                  # 4 h-tiles
SBLK = 512                     # s-block width
NSB = S // SBLK                # 8 s-blocks per batch
KT = H // 128                  # 4 k-tiles for the hidden projection

F8PAIRS = int(os.environ.get("ATTN_F8PAIRS", "4"))   # e-tile pairs in fp8-DR
GROUP_SB = int(os.environ.get("ATTN_GROUP_SB", "2"))
GP_ET = int(os.environ.get("ATTN_GP_ET", "0"))       # ctx e-tiles on gpsimd
E8T = 2 * F8PAIRS              # e-tiles fed as fp8
EW = ET * SBLK                 # fp16 elems per (partition, s-block)
E8W = E8T * SBLK               # fp8 elems per (partition, s-block)

TRACE = False          # set by test harness; harness-default off
LAST_RESULTS = None    # last BassKernelResults (for profiling in test.py)

_NC_CACHE = {}


def _groups():
    groups = []
    pos = 0
    while pos < NSB - 2:
        groups.append(list(range(pos, pos + GROUP_SB)))
        pos += GROUP_SB
    while pos < NSB:
        groups.append([pos])
        pos += 1
    return groups


def _build():
    nc = bacc.Bacc("TRN2", target_bir_lowering=False, debug=False)

    CW = (KT + 1) + KT * BPC            # bshift | hidT, packed (128, CW) f32
    # s-block-major packed layouts: [b, sb, p, t*SBLK+s]
    encT = nc.dram_tensor("encT", [BPC, NSB, 128, EW], F16,
                          kind="ExternalInput").ap()
    if E8T:
        encT8 = nc.dram_tensor("encT8", [BPC, NSB, 128, E8W], F8,
                               kind="ExternalInput").ap()
        We8_d = nc.dram_tensor(
            "We8", [128, E8T * H], F8, kind="ExternalInput").ap()
    We_d = nc.dram_tensor("We", [128, ET * H], F16, kind="ExternalInput").ap()
    V_d = nc.dram_tensor("V128", [128, HT * 128], F16, kind="ExternalInput").ap()
    Wh_d = nc.dram_tensor("Wh16", [128, KT * H], F16, kind="ExternalInput").ap()
    cst_d = nc.dram_tensor("consts", [128, CW], F32, kind="ExternalInput").ap()
    ctx_d = nc.dram_tensor("ctx", [BPC, E], F32, kind="ExternalOutput").ap()

    groups = _groups()
    NG = len(groups)
    DVE_ET = ET - GP_ET

    with tile.TileContext(nc) as tc, ExitStack() as ctx:
        cpool = ctx.enter_context(tc.tile_pool(name="consts", bufs=1))
        epool = ctx.enter_context(tc.tile_pool(name="enc", bufs=4))
        e8pool = ctx.enter_context(tc.tile_pool(name="enc8", bufs=3))
        tpool = ctx.enter_context(tc.tile_pool(name="tanh", bufs=2))
        wpool = ctx.enter_context(tc.tile_pool(name="wexp", bufs=2))
        jpool = ctx.enter_context(tc.tile_pool(name="junk", bufs=2))
        spool = ctx.enter_context(tc.tile_pool(name="stats", bufs=1))
        proj_bufs = 1 if GROUP_SB >= 4 else 2
        pp = ctx.enter_context(tc.tile_pool(name="pproj", bufs=proj_bufs, space="PSUM"))
        pl = ctx.enter_context(tc.tile_pool(name="plog", bufs=2, space="PSUM"))
        ph_pool = ctx.enter_context(tc.tile_pool(name="phid", bufs=1, space="PSUM"))

        # ---- PE warm-up: dummy matmuls while DMAs land (HAM -> K=8/8) ----
        wlhs = cpool.tile([128, 128], F16)
        wrhs = cpool.tile([128, 256], F16)
        nc.vector.memset(wlhs[:], 0.0)
        nc.vector.memset(wrhs[:], 0.0)
        wps = ph_pool.tile([128, 256], F32, name="warm", tag="ph")
        for _ in range(18):
            nc.tensor.matmul(wps[:], wlhs[:], wrhs[:], start=True, stop=True)

        # ---- constants: packed DMAs (small gate-the-warmup one first) ----
        cst_sb = cpool.tile([128, CW], F32)
        nc.scalar.dma_start(cst_sb[:], cst_d)
        if E8T:
            We8_sb = cpool.tile([128, E8T * H], F8)
            nc.scalar.dma_start(We8_sb[:], We8_d)
            We8_v = We8_sb[:].rearrange("p (t c) -> p t c", t=E8T)
        We_sb = cpool.tile([128, ET * H], F16)          # per e-tile: (128, 512)
        nc.scalar.dma_start(We_sb[:, 0:2 * H], We_d[:, 0:2 * H])
        Wh_sb = cpool.tile([128, KT * H], F16)
        nc.scalar.dma_start(Wh_sb[:], Wh_d)
        nc.scalar.dma_start(We_sb[:, 2 * H:], We_d[:, 2 * H:])
        V_sb = cpool.tile([128, HT * 128], F16)
        nc.scalar.dma_start(V_sb[:], V_d)
        bsh_sb = cst_sb[:, 0:KT + 1]
        hidT16 = cpool.tile([128, KT * BPC], F16)
        nc.vector.tensor_copy(hidT16[:], cst_sb[:, KT + 1:KT + 1 + KT * BPC])

        # ---- hidden projection: hid_sb[:, h*BPC + b] = (hidden @ Wh + b)[b, h-tile]
        hid_sb = spool.tile([128, HT * BPC], F32)
        for h in range(HT):
            ph = ph_pool.tile([128, BPC], F32, name="ph")
            for k in range(KT):
                nc.tensor.matmul(
                    ph[:],
                    Wh_sb[:, k * H + h * 128: k * H + (h + 1) * 128],
                    hidT16[:, k * BPC:(k + 1) * BPC],
                    start=(k == 0), stop=(k == KT - 1),
                )
            nc.vector.tensor_scalar_add(
                hid_sb[:, h * BPC:(h + 1) * BPC], ph[:], bsh_sb[:, h:h + 1])

        # ---- stats accumulators (per-group ctx partials; DVE/GP separate)
        zslots = spool.tile([128, BPC * NSB], F32)
        cslots = spool.tile([128, BPC * DVE_ET * NG], F32)
        if GP_ET:
            gslots = spool.tile([128, BPC * GP_ET * NG], F32)

        ctx_red = spool.tile([128, BPC * ET], F32)
        zred = spool.tile([128, BPC], F32)
        zrec = spool.tile([128, BPC], F32)
        ctx_fin = spool.tile([128, BPC * ET], F32)

        for b in range(BPC):
            for g, sbs in enumerate(groups):
                gsb = len(sbs)
                gw = gsb * SBLK
                g0 = sbs[0]
                # fp8 stream feeds the PE as early as possible
                if E8T:
                    enc8g = e8pool.tile([128, GROUP_SB * E8W], F8,
                                        name="enc8g", tag="enc8g")
                    if b == 0 and g == 0:
                        # split so the first h-iteration can start earlier
                        for i in range(gsb):
                            for t0, tn in ((0, E8T // 2), (E8T // 2, E8T // 2)):
                                nc.sync.dma_start(
                                    enc8g[:, i * E8W + t0 * SBLK:
                                          i * E8W + (t0 + tn) * SBLK],
                                    encT8[b, sbs[i], :,
                                          t0 * SBLK:(t0 + tn) * SBLK],
                                )
                    else:
                        nc.sync.dma_start(
                            enc8g[:, 0:gsb * E8W].rearrange(
                                "p (i f) -> p i f", i=gsb),
                            encT8[b].rearrange("s p f -> p s f")[
                                :, g0:g0 + gsb, :],
                        )
                    enc8g_v = enc8g[:, 0:gsb * E8W].rearrange(
                        "p (i t s) -> p i t s", i=gsb, t=E8T)
                # fp16 stream feeds the DVE/GP ctx pass (and fp16 matmuls)
                encg = epool.tile([128, GROUP_SB * EW], F16, name="encg",
                                  tag="encg")
                nc.scalar.dma_start(
                    encg[:, 0:gsb * EW].rearrange("p (i f) -> p i f", i=gsb),
                    encT[b].rearrange("s p f -> p s f")[:, g0:g0 + gsb, :],
                )
                encg_v = encg[:, 0:gsb * EW].rearrange(
                    "p (i t s) -> p i t s", i=gsb, t=ET)

                # big projection + tanh, h-tile at a time
                tanh_t = {}
                for h in range(HT):
                    proj = {}
                    for i in range(gsb):
                        proj[i] = pp.tile([128, SBLK], F32, name=f"proj_{i}")
                    for ep in range(F8PAIRS):
                        lhsT = We8_v[:, 2 * ep:2 * ep + 2,
                                     h * 128:(h + 1) * 128]
                        for i in range(gsb):
                            nc.tensor.matmul(
                                proj[i][:], lhsT,
                                enc8g_v[:, i, 2 * ep:2 * ep + 2, :],
                                start=(ep == 0),
                                stop=(ep == F8PAIRS - 1 and E8T == ET),
                                perf_mode=mybir.MatmulPerfMode.DoubleRow,
                            )
                    for e in range(E8T, ET):
                        lhs = We_sb[:, e * H + h * 128: e * H + (h + 1) * 128]
                        for i in range(gsb):
                            nc.tensor.matmul(
                                proj[i][:], lhs, encg_v[:, i, e, :],
                                start=(e == 0), stop=(e == ET - 1),
                            )
                    for i in range(gsb):
                        tt = tpool.tile([128, SBLK], F16, name=f"tanh_{h}_{i}")
                        nc.scalar.activation(
                            tt[:], proj[i][:], mybir.ActivationFunctionType.Tanh,
                            bias=hid_sb[:, h * BPC + b: h * BPC + b + 1],
                        )
                        tanh_t[(h, i)] = tt

                # logits (broadcast across partitions) + exp + Z accum
                wg = wpool.tile([128, GROUP_SB * SBLK], BF16, name="wg")
                lg = {}
                for i in range(gsb):
                    lg[i] = pl.tile([128, SBLK], F32, name=f"logits_{i}",
                                    bufs=1)
                for h in range(HT):
                    for i in range(gsb):
                        nc.tensor.matmul(
                            lg[i][:], V_sb[:, h * 128:(h + 1) * 128],
                            tanh_t[(h, i)][:],
                            start=(h == 0), stop=(h == HT - 1),
                        )
                for i, sb in enumerate(sbs):
                    nc.scalar.activation(
                        wg[:, i * SBLK:(i + 1) * SBLK], lg[i][:],
                        mybir.ActivationFunctionType.Exp,
                        bias=bsh_sb[:, KT:KT + 1],
                        accum_out=zslots[:, b * NSB + sb: b * NSB + sb + 1],
                    )
                wg_v = wg[:, 0:gw].rearrange("p (i s) -> p i s", i=gsb)

                # context accumulation: fused multiply+accumulate per e-tile,
                # whole group in one 3D-AP op; split DVE / GPSIMD by e-tile.
                for e in range(ET):
                    if e < DVE_ET:
                        eng, name = nc.vector, "junk"
                        col = (b * DVE_ET + e) * NG + g
                        acc = cslots
                    else:
                        eng, name = nc.gpsimd, "gjunk"
                        col = (b * GP_ET + (e - DVE_ET)) * NG + g
                        acc = gslots
                    jt = jpool.tile([128, GROUP_SB * SBLK], F16, name=name)
                    eng.scalar_tensor_tensor(
                        jt[:, 0:gw].rearrange("p (i s) -> p i s", i=gsb),
                        encg_v[:, :, e, :], 1.0, wg_v,
                        mybir.AluOpType.mult, mybir.AluOpType.mult,
                        accum_out=acc[:, col:col + 1],
                    )

            # finalize this batch: ctx = (sum_g ctx_partial) / Z
            nc.vector.tensor_reduce(
                ctx_red[:, b * ET:b * ET + DVE_ET],
                cslots[:, b * DVE_ET * NG:(b + 1) * DVE_ET * NG].rearrange(
                    "p (q s) -> p q s", s=NG),
                axis=mybir.AxisListType.X, op=mybir.AluOpType.add)
            if GP_ET:
                nc.vector.tensor_reduce(
                    ctx_red[:, b * ET + DVE_ET:(b + 1) * ET],
                    gslots[:, b * GP_ET * NG:(b + 1) * GP_ET * NG].rearrange(
                        "p (q s) -> p q s", s=NG),
                    axis=mybir.AxisListType.X, op=mybir.AluOpType.add)
            nc.vector.tensor_reduce(
                zred[:, b:b + 1],
                zslots[:, b * NSB:(b + 1) * NSB].rearrange(
                    "p (q s) -> p q s", s=NSB),
                axis=mybir.AxisListType.X, op=mybir.AluOpType.add)
            nc.vector.reciprocal(zrec[:, b:b + 1], zred[:, b:b + 1])
            nc.vector.tensor_scalar_mul(
                ctx_fin[:, b * ET:(b + 1) * ET],
                ctx_red[:, b * ET:(b + 1) * ET], zrec[:, b:b + 1])
            nc.sync.dma_start(
                ctx_d[b].rearrange("(e p) -> p e", p=128),
                ctx_fin[:, b * ET:(b + 1) * ET])

    nc.compile()
    return nc


def kernel(hidden, encoder_outputs, W_attn, b_attn, v):
    global LAST_RESULTS
    hidden = np.asarray(hidden, dtype=np.float32)
    encoder_outputs = np.asarray(encoder_outputs, dtype=np.float32)
    W_attn = np.asarray(W_attn, dtype=np.float32)
    b_attn = np.asarray(b_attn, dtype=np.float32)
    v = np.asarray(v, dtype=np.float32)

    key = (F8PAIRS, GROUP_SB, GP_ET)
    if key not in _NC_CACHE:
        _NC_CACHE[key] = _build()
    nc = _NC_CACHE[key]

    # SBUF-layout packed constants (partition dim = 128 rows)
    WeT = W_attn[H:].reshape(ET, 128, H).transpose(1, 0, 2).reshape(128, ET * H)
    We_f = np.ascontiguousarray(WeT.astype(np.float16))
    V128 = np.ascontiguousarray(np.broadcast_to(
        v.reshape(HT, 128, 1).transpose(1, 0, 2), (128, HT, 128)
    ).reshape(128, HT * 128).astype(np.float16))
    Wh16 = np.ascontiguousarray(W_attn[:H].reshape(KT, 128, H).transpose(
        1, 0, 2).reshape(128, KT * H).astype(np.float16))
    shift = float(np.abs(v).sum())
    bsh = np.zeros((128, KT + 1), dtype=np.float32)
    bsh[:, :KT] = b_attn.reshape(KT, 128).T
    bsh[:, KT] = -shift
    if E8T:
        We8 = np.ascontiguousarray(WeT[:, 0:E8T * H].astype(E4M3))

    in_maps = []
    for c in range(NCORES):
        sl = slice(c * BPC, (c + 1) * BPC)
        # [b, sb, p, t, s] s-block-major pack (8KB contiguous per partition)
        encP = encoder_outputs[sl].reshape(
            BPC, NSB, SBLK, ET, 128).transpose(0, 1, 4, 3, 2)
        hidT_pack = np.ascontiguousarray(
            hidden[sl].T.reshape(KT, 128, BPC).transpose(1, 0, 2)
        ).reshape(128, KT * BPC)
        consts = np.ascontiguousarray(
            np.concatenate([bsh, hidT_pack], axis=1, dtype=np.float32))
        m = {
            "encT": np.ascontiguousarray(encP.astype(np.float16)).reshape(
                BPC, NSB, 128, EW),
            "We": We_f, "V128": V128, "consts": consts, "Wh16": Wh16,
        }
        if E8T:
            m["encT8"] = np.ascontiguousarray(
                encP[:, :, :, 0:E8T, :].astype(E4M3)).reshape(
                    BPC, NSB, 128, E8W)
            m["We8"] = We8
        in_maps.append(m)

    res = run_bass_kernel_spmd(
        nc, in_maps, core_ids=list(range(NCORES)), trace=TRACE)
    LAST_RESULTS = res

    out = np.empty((B, 1, E), dtype=np.float32)
    for c in range(NCORES):
        out[c * BPC:(c + 1) * BPC, 0, :] = res.results[c]["ctx"]
    return out
